# revision 33
# baseline (speedup 1.0000x reference)
"""Trainium2 Bass kernel for nn_ChannelSpatialAttention2 (dense_cnn).

Data-parallel over batch: 16 samples / 8 cores = 2 samples per core, no
cross-core communication.  Per-sample dataflow (channel-major layout
(128 ch, 16384 px), px = h*128 + w):

  1. HWDGE DMA-cast loads f_vi/f_ir fp32->bf16 (fv on SP queue, fi on DVE
     queue); pooled channel sums ride on vector tensor_scalar accum_out.
  2. Tiny MLP (1x1 convs + BN folded on host) -> channel weights a.
  3. conv1 folded:  g_pre = (Wv + Wi*diag(a)) @ f_vi + (Wi + Wv*diag(a)) @ f_ir
     (the complementary blend never materializes).  BN+ReLU fused into the
     PSUM->SBUF activation with per-partition scale/bias; avg-pool rides on
     accum_out.  Matmuls ordered to amortize Ldweights.
  4. Per quarter: DMA-xbar transpose -> gT; channel-max halving tree on the
     transposed free dim -> maxpad cols; per-tile channel-mean via one-column
     matmuls -> psum (W,H) map; partial pixel-max tree in place on the dead
     g quarter (keeps the big serial tree off the critical path).
  5. 7x7 spatial conv = 14 accumulating matmuls against host-built banded
     matrices (H-shifts via stationary-operand slicing).
  6. wgt field: rank-1 outer product sigma(ca x sa) via K=1 matmuls,
     out = diag(a)@f_vi + I@f_ir + diag(1-a)@(t*(f_vi-f_ir)) accumulated in
     PSUM, evacuated alternately by scalar/vector engines, DMA'd out.

  All constants ride in 2 packed DMAs.  The two samples are emitted
  stage-interleaved (L0 L1 MLP0 C0 T0 MLP1 C1 B0 T1 B1) so PE/DVE/Act/DMA
  stay busy across the serial per-sample phases.
"""

import sys

if '/opt/trn_rl_repo' not in sys.path:
    sys.path.insert(0, '/opt/trn_rl_repo')

import numpy as np
import ml_dtypes

import concourse.bacc as bacc
import concourse.mybir as mybir
import concourse.tile as tile
import concourse.bass_utils as bass_utils

EPS = 1e-5
C = 128
N, H, W = 16, 128, 128
P = H * W            # 16384 pixels per sample
NCORES = 8
import os
SPC = int(os.environ.get('KSPC', N // NCORES))

BF16 = mybir.dt.bfloat16
F32 = mybir.dt.float32
AL = mybir.AluOpType
AF = mybir.ActivationFunctionType
AX = mybir.AxisListType

_cache = {}

# packed bf16 const layout: wvi(128) wir(128) eye(128) bmat(14*128) one(1)
CB_W = 128 + 128 + 128 + 14 * 128 + 1
# packed f32 const layout: l1v(64) l1i(64) b1(1) l2(128) b2(1) sc(1) bc(1)
#                          c1a(8) c1m(8) c2r(128)
CF_W = 64 + 64 + 1 + 128 + 1 + 1 + 1 + 8 + 8 + 128
QP = 4096            # load-chunk pixels


def _build_program():
    nc = bacc.Bacc("TRN2", target_bir_lowering=False, debug=False,
                   enable_asserts=False, num_devices=NCORES)

    d_fvi = nc.dram_tensor("f_vi", (SPC, C, P), F32, kind="ExternalInput").ap()
    d_fir = nc.dram_tensor("f_ir", (SPC, C, P), F32, kind="ExternalInput").ap()
    d_out = nc.dram_tensor("out", (SPC, C, P), F32, kind="ExternalOutput").ap()
    d_cb = nc.dram_tensor("cb", (C, CB_W), BF16, kind="ExternalInput").ap()
    d_cf = nc.dram_tensor("cf", (C, CF_W), F32, kind="ExternalInput").ap()

    with tile.TileContext(nc) as tc:
        with (
            tc.tile_pool(name="wts", bufs=1) as wts,
            tc.tile_pool(name="io", bufs=8) as io,
            tc.tile_pool(name="gbuf", bufs=4) as gbuf,
            tc.tile_pool(name="gtb", bufs=2) as gtb,
            tc.tile_pool(name="sfl", bufs=1) as sfl,
            tc.tile_pool(name="sm", bufs=2) as sm,
            tc.tile_pool(name="bl", bufs=3) as bl,
            tc.tile_pool(name="ob", bufs=3) as obp,
            tc.tile_pool(name="ps", bufs=3, space="PSUM") as ps,
            tc.tile_pool(name="psm", bufs=2, space="PSUM") as psmp,
        ):
            # ---- constant loads (2 packed DMAs) ----
            cb = wts.tile([C, CB_W], BF16)
            cf = wts.tile([C, CF_W], F32)
            nc.sync.dma_start(cb[:], d_cb[:])
            nc.scalar.dma_start(cf[:], d_cf[:])
            wvi = cb[:, 0:128]
            wir = cb[:, 128:256]
            eye = cb[:, 256:384]
            bmat = cb[:, 384:384 + 14 * 128].rearrange("p (m c) -> p m c", c=128)
            one_col = cb[:, CB_W - 1:CB_W]
            l1v = cf[0:C, 0:64]
            l1i = cf[0:C, 64:128]
            b1 = cf[0:64, 128:129]
            l2 = cf[0:64, 129:257]
            b2 = cf[0:C, 257:258]
            sc = cf[0:C, 258:259]
            bc = cf[0:C, 259:260]
            c1a = cf[0:C, 260:268]
            c1m = cf[0:C, 268:276]
            c2r = cf[0:8, 276:404]

            st = [dict() for _ in range(SPC)]   # per-sample state

            def FV(s, sl):
                t = st[s]['fvh'][sl.start // QP]
                o = sl.start % QP
                return t[:, o:o + sl.stop - sl.start]

            def FI(s, sl):
                t = st[s]['fih'][sl.start // QP]
                o = sl.start % QP
                return t[:, o:o + sl.stop - sl.start]

            # ---------------- stage L: loads + GAP ----------------
            def stage_L(s):
                d = st[s]
                d['fvh'] = [io.tile([C, QP], BF16, tag="fv", name=f"fv{s}_{i}")
                            for i in range(4)]
                d['fih'] = [io.tile([C, QP], BF16, tag="fi", name=f"fi{s}_{i}")
                            for i in range(4)]
                for k in range(4):
                    sl = slice(k * QP, (k + 1) * QP)
                    nc.gpsimd.dma_start(FV(s, sl), d_fvi[s][:, sl])
                    nc.gpsimd.dma_start(FI(s, sl), d_fir[s][:, sl])
                pvp = sm.tile([C, 4], F32, tag="pvp", name=f"pvp{s}")
                pip = sm.tile([C, 4], F32, tag="pip", name=f"pip{s}")
                for k in range(4):
                    sl = slice(k * QP, (k + 1) * QP)
                    nc.vector.tensor_scalar(FV(s, sl), FV(s, sl), 1.0, 0.0,
                                            AL.mult, AL.add,
                                            accum_out=pvp[:, k:k + 1])
                    nc.vector.tensor_scalar(FI(s, sl), FI(s, sl), 1.0, 0.0,
                                            AL.mult, AL.add,
                                            accum_out=pip[:, k:k + 1])
                sv = sm.tile([C, 1], F32, tag="sv", name=f"sv{s}")
                si = sm.tile([C, 1], F32, tag="si", name=f"si{s}")
                nc.vector.reduce_sum(sv[:], pvp[:], axis=AX.X)
                nc.vector.reduce_sum(si[:], pip[:], axis=AX.X)
                d['sv'], d['si'] = sv, si

            # ---------------- stage MLP: a -> per-sample weights ----------------
            def stage_MLP(s):
                d = st[s]
                ps1 = ps.tile([C, 1024], F32, tag="ps", name=f"ps1_{s}")
                nc.tensor.matmul(ps1[0:64, 0:1], l1v, d['sv'][:], start=True, stop=False)
                nc.tensor.matmul(ps1[0:64, 0:1], l1i, d['si'][:], start=False, stop=True)
                h1 = sm.tile([64, 1], F32, tag="h1", name=f"h1_{s}")
                nc.scalar.activation(h1[:], ps1[0:64, 0:1], AF.Relu, bias=b1)
                ps2 = ps.tile([C, 1024], F32, tag="ps", name=f"ps2_{s}")
                nc.tensor.matmul(ps2[0:C, 0:1], l2, h1[:], start=True, stop=True)
                a_col = sm.tile([C, 1], F32, tag="a_col", name=f"a_col{s}")
                nc.scalar.activation(a_col[:], ps2[0:C, 0:1], AF.Sigmoid, bias=b2)
                oma = sm.tile([C, 1], F32, tag="oma", name=f"oma{s}")
                nc.vector.tensor_scalar(oma[:], a_col[:], -1.0, 1.0, AL.mult, AL.add)
                lv = sm.tile([C, C], BF16, tag="lv", name=f"lv{s}")
                li = sm.tile([C, C], BF16, tag="li", name=f"li{s}")
                dga = sm.tile([C, C], BF16, tag="dga", name=f"dga{s}")
                dgo = sm.tile([C, C], BF16, tag="dgo", name=f"dgo{s}")
                nc.vector.scalar_tensor_tensor(lv[:], wir, a_col[:], wvi, AL.mult, AL.add)
                nc.vector.scalar_tensor_tensor(li[:], wvi, a_col[:], wir, AL.mult, AL.add)
                nc.vector.tensor_scalar(dga[:], eye, a_col[:], 0.0, AL.mult, AL.add)
                nc.vector.tensor_scalar(dgo[:], eye, oma[:], 0.0, AL.mult, AL.add)
                d['lv'], d['li'], d['dga'], d['dgo'] = lv, li, dga, dgo

            # ---------------- stage C: conv1 + per-quarter maps ----------------
            def stage_C(s):
                d = st[s]
                lv, li = d['lv'], d['li']
                avp = sm.tile([C, 16], F32, tag="avp", name=f"avp{s}")
                rem = sm.tile([C, 256], BF16, tag="rem", name=f"rem{s}")
                maxpad = sm.tile([128, 134], BF16, tag="maxpad", name=f"maxpad{s}")
                sumpad = sm.tile([128, 134], BF16, tag="sumpad", name=f"sumpad{s}")
                nc.vector.memset(maxpad[:, 0:3], 0.0)
                nc.vector.memset(maxpad[:, 131:134], 0.0)
                nc.vector.memset(sumpad[:, 0:3], 0.0)
                nc.vector.memset(sumpad[:, 131:134], 0.0)
                psmm = psmp.tile([C, 512], F32, tag="psm", name=f"psmm{s}")
                gqs = {}

                def chmean(pt):
                    # channel-mean of the 8 image rows of tile pt (W,H map);
                    # delayed one tile so PE never waits on the evac
                    gq_, half_ = gqs[pt // 2], pt % 2
                    for r in range(8):
                        hh = pt * 8 + r
                        ro = (half_ * 8 + r) * 128
                        nc.tensor.matmul(psmm[:, hh:hh + 1],
                                         gq_[:, ro:ro + 128], one_col,
                                         start=True, stop=True)

                def halfq(hq):
                    # transpose + channel-max tree + partial pixel-max on the
                    # (now dead) half-quarter hq
                    gq_ = gqs[hq]
                    gt = gtb.tile([128, 2048], BF16, tag="gt", name=f"gt{s}_{hq}")
                    gt3 = gt[:].rearrange("p (h c) -> p h c", c=128)
                    nc.sync.dma_start_transpose(gt3, gq_[:])
                    for w_ in (64, 32):
                        nc.vector.tensor_tensor(
                            out=gt3[:, :, 0:w_], in0=gt3[:, :, 0:w_],
                            in1=gt3[:, :, w_:2 * w_], op=AL.max)
                    nc.vector.reduce_max(
                        maxpad[:, 3 + hq * 16:3 + hq * 16 + 16],
                        gt3[:, :, 0:32], axis=AX.X)
                    for w_ in (1024, 512, 256):
                        nc.vector.tensor_tensor(
                            out=gq_[:, 0:w_], in0=gq_[:, 0:w_],
                            in1=gq_[:, w_:2 * w_], op=AL.max)
                    if hq == 0:
                        nc.vector.tensor_copy(rem[:], gq_[:, 0:256])
                    else:
                        nc.vector.tensor_tensor(out=rem[:], in0=rem[:],
                                                in1=gq_[:, 0:256], op=AL.max)

                for pt in range(16):
                    base = pt * 1024
                    hq, half = pt // 2, pt % 2
                    if half == 0:
                        gqs[hq] = gbuf.tile([C, 2048], BF16, tag="gq",
                                            name=f"gq{s}_{hq}")
                    pg = ps.tile([C, 1024], F32, tag="ps", name=f"pg{s}_{pt}")
                    s0 = slice(base, base + 512)
                    s1 = slice(base + 512, base + 1024)
                    # lv,lv,li,li order: 2 weight loads per tile instead of 4
                    nc.tensor.matmul(pg[:, 0:512], lv[:], FV(s, s0), start=True, stop=False)
                    nc.tensor.matmul(pg[:, 512:1024], lv[:], FV(s, s1), start=True, stop=False)
                    nc.tensor.matmul(pg[:, 0:512], li[:], FI(s, s0), start=False, stop=True)
                    nc.tensor.matmul(pg[:, 512:1024], li[:], FI(s, s1), start=False, stop=True)
                    go = half * 1024
                    nc.scalar.activation(gq[:, go:go + 1024] if False else gqs[hq][:, go:go + 1024], pg[:],
                                         AF.Relu, bias=bc, scale=sc,
                                         accum_out=avp[:, pt:pt + 1])
                    if pt >= 1:
                        chmean(pt - 1)
                    if pt >= 3 and pt % 2 == 1:
                        halfq(pt // 2 - 1)
                    yield
                chmean(15)
                halfq(7)
                # evacuate channel-mean map (w, h); 1/128 scale folded in bmat
                nc.scalar.activation(sumpad[:, 3:131], psmm[:, 0:128], AF.Copy)
                d['rem'], d['avp'] = rem, avp
                d['maxpad'], d['sumpad'] = maxpad, sumpad

            # ---------------- stage T: pools, CA MLP, SA conv ----------------
            def stage_T(s):
                d = st[s]
                mx = sm.tile([C, 1], F32, tag="mx", name=f"mx{s}")
                nc.vector.reduce_max(mx[:], d['rem'][:], axis=AX.X)
                avs = sm.tile([C, 1], F32, tag="avs", name=f"avs{s}")
                nc.vector.reduce_sum(avs[:], d['avp'][:], axis=AX.X)
                # SpatialAttention 7x7 conv as banded matmuls (no CA deps)
                pss = ps.tile([C, 1024], F32, tag="ps", name=f"pss{s}")
                first = True
                for chn, pad in ((0, d['sumpad']), (1, d['maxpad'])):
                    for dy in range(7):
                        nc.tensor.matmul(pss[0:128, 0:128], pad[:, dy:dy + 128],
                                         bmat[:, chn * 7 + dy, :],
                                         start=first, stop=(chn == 1 and dy == 6))
                        first = False
                sa_hw = sm.tile([128, 128], BF16, tag="sa_hw", name=f"sa_hw{s}")
                nc.scalar.activation(sa_hw[:], pss[0:128, 0:128], AF.Sigmoid)
                # [65, 8192]: px 0..8191 on partition 0, px 8192.. on 64
                # (the only extra legal moving-operand base partition)
                sa_fl = sfl.tile([65, P // 2], BF16, tag="sa_fl", name=f"sa_fl{s}")
                nc.sync.dma_start(sa_fl[0:1, :], sa_hw[0:64, :])
                nc.sync.dma_start(sa_fl[64:65, :], sa_hw[64:128, :])
                d['sa_fl'] = sa_fl
                yield
                # ChannelAttention MLP (cross-engine chain, interleaved by caller)
                psa = ps.tile([C, 1024], F32, tag="ps", name=f"psa{s}")
                nc.tensor.matmul(psa[0:8, 0:1], c1a, avs[:], start=True, stop=True)
                psm2 = ps.tile([C, 1024], F32, tag="ps", name=f"psm2{s}")
                nc.tensor.matmul(psm2[0:8, 0:1], c1m, mx[:], start=True, stop=True)
                ha = sm.tile([8, 1], F32, tag="ha", name=f"ha{s}")
                nc.scalar.activation(ha[:], psa[0:8, 0:1], AF.Relu)
                hm = sm.tile([8, 1], F32, tag="hm", name=f"hm{s}")
                nc.scalar.activation(hm[:], psm2[0:8, 0:1], AF.Relu)
                yield
                psr = ps.tile([C, 1024], F32, tag="ps", name=f"psr{s}")
                nc.tensor.matmul(psr[0:1, 0:C], ha[:], c2r, start=True, stop=False)
                nc.tensor.matmul(psr[0:1, 0:C], hm[:], c2r, start=False, stop=True)
                yield
                ca = sm.tile([1, C], BF16, tag="ca", name=f"ca{s}")
                nc.scalar.activation(ca[:], psr[0:1, 0:C], AF.Sigmoid)
                d['ca'] = ca
                yield

            # ---------------- stage B: blend + stores ----------------
            def stage_B(s):
                d = st[s]
                dga, dgo, sa_fl = d['dga'], d['dgo'], d['sa_fl']
                for bt in range(16):
                    ca = d['ca']
                    base = bt * 1024
                    sl0 = slice(base, base + 512)
                    sl1 = slice(base + 512, base + 1024)
                    slf = slice(base, base + 1024)
                    pw = ps.tile([C, 1024], F32, tag="ps", name=f"pw{s}_{bt}")
                    jp, off = (0, base) if base < P // 2 else (64, base - P // 2)
                    nc.tensor.matmul(pw[:, 0:512], ca[:],
                                     sa_fl[jp:jp + 1, off:off + 512], start=True, stop=True)
                    nc.tensor.matmul(pw[:, 512:1024], ca[:],
                                     sa_fl[jp:jp + 1, off + 512:off + 1024], start=True, stop=True)
                    tt = bl.tile([C, 1024], BF16, tag="tt", name=f"tt{s}_{bt}")
                    nc.scalar.activation(tt[:], pw[:], AF.Sigmoid)
                    dt = bl.tile([C, 1024], BF16, tag="dt", name=f"dt{s}_{bt}")
                    nc.vector.scalar_tensor_tensor(dt[:], FV(s, slf), 1.0, FI(s, slf),
                                                   AL.mult, AL.subtract)
                    nc.vector.tensor_tensor(out=dt[:], in0=dt[:], in1=tt[:], op=AL.mult)
                    pb = ps.tile([C, 1024], F32, tag="ps", name=f"pb{s}_{bt}")
                    nc.tensor.matmul(pb[:, 0:512], dga[:], FV(s, sl0), start=True, stop=False)
                    nc.tensor.matmul(pb[:, 512:1024], dga[:], FV(s, sl1), start=True, stop=False)
                    if bt % 2 == 0:
                        nc.tensor.matmul(pb[:, 0:512], eye, FI(s, sl0), start=False, stop=False)
                        nc.tensor.matmul(pb[:, 512:1024], eye, FI(s, sl1), start=False, stop=False)
                    nc.tensor.matmul(pb[:, 0:512], dgo[:], dt[:, 0:512], start=False, stop=True)
                    nc.tensor.matmul(pb[:, 512:1024], dgo[:], dt[:, 512:1024], start=False, stop=True)
                    ob = obp.tile([C, 1024], F32, tag="ob", name=f"ob{s}_{bt}")
                    if bt % 2 == 0:
                        nc.scalar.copy(ob[:], pb[:])
                    else:
                        nc.vector.scalar_tensor_tensor(ob[:], pb[:], 1.0, FI(s, slf),
                                                       AL.mult, AL.add)
                    nc.sync.dma_start(d_out[s][:, slf], ob[:])
                    yield

            # ---------------- pipelined emission ----------------
            def drain(gen):
                for _ in gen:
                    pass

            def interleave(serial, filler, ratio):
                # emit `ratio` filler steps between serial-chain hops
                for _ in serial:
                    for _ in range(ratio):
                        if next(filler, 'END') == 'END':
                            break
                drain(filler)

            if SPC == 2:
                stage_L(0)
                stage_L(1)
                stage_MLP(0)
                drain(stage_C(0))
                stage_MLP(1)
                interleave(stage_T(0), stage_C(1), 3)
                interleave(stage_T(1), stage_B(0), 3)
                drain(stage_B(1))
            else:
                for s in range(SPC):
                    stage_L(s)
                    stage_MLP(s)
                    drain(stage_C(s))
                    drain(stage_T(s))
                    drain(stage_B(s))

    nc.compile()
    return nc


def _host_consts(ca1_w, ca1_b, bn_a_g, bn_a_b, bn_a_m, bn_a_v,
                 ca2_w, ca2_b, bn_b_g, bn_b_b, bn_b_m, bn_b_v,
                 conv1_w, conv1_b, bn_c_g, bn_c_b, bn_c_m, bn_c_v,
                 chatt_w1, chatt_w2, sa_w):
    bf = ml_dtypes.bfloat16
    f = np.float32
    k_a = bn_a_g / np.sqrt(bn_a_v + EPS)
    w1 = ca1_w * k_a[:, None]
    b1 = (ca1_b - bn_a_m) * k_a + bn_a_b
    k_b = bn_b_g / np.sqrt(bn_b_v + EPS)
    w2 = ca2_w * k_b[:, None]
    b2 = (ca2_b - bn_b_m) * k_b + bn_b_b
    s_c = bn_c_g / np.sqrt(bn_c_v + EPS)
    b_c = (conv1_b - bn_c_m) * s_c + bn_c_b
    bmat = np.zeros((14, 128, 128), np.float32)
    for chn in range(2):
        scale = (1.0 / 128.0) if chn == 0 else 1.0
        for dy in range(7):
            for dx in range(7):
                off = dx - 3          # w' - w
                v = sa_w[0, chn, dy, dx] * scale
                if off >= 0:
                    idx = np.arange(0, 128 - off)
                    bmat[chn * 7 + dy, idx + off, idx] = v
                else:
                    idx = np.arange(-off, 128)
                    bmat[chn * 7 + dy, idx + off, idx] = v

    cbm = np.zeros((C, CB_W), np.float32)
    cbm[:, 0:128] = conv1_w[:, :C].T
    cbm[:, 128:256] = conv1_w[:, C:].T
    cbm[:, 256:384] = np.eye(C)
    cbm[:, 384:384 + 14 * 128] = np.transpose(bmat, (1, 0, 2)).reshape(C, 14 * 128)
    cbm[:, CB_W - 1] = 1.0

    cfm = np.zeros((C, CF_W), np.float32)
    cfm[:, 0:64] = (w1[:, :C] / P).T
    cfm[:, 64:128] = (w1[:, C:] / P).T
    cfm[0:64, 128] = b1
    cfm[0:64, 129:257] = w2.T
    cfm[:, 257] = b2
    cfm[:, 258] = s_c
    cfm[:, 259] = b_c
    cfm[:, 260:268] = (chatt_w1 / P).T
    cfm[:, 268:276] = chatt_w1.T
    cfm[0:8, 276:404] = chatt_w2.T

    return {
        "cb": cbm.astype(bf),
        "cf": cfm.astype(f),
    }


def kernel(f_vi, f_ir, ca1_w, ca1_b, bn_a_g, bn_a_b, bn_a_m, bn_a_v,
           ca2_w, ca2_b, bn_b_g, bn_b_b, bn_b_m, bn_b_v,
           conv1_w, conv1_b, bn_c_g, bn_c_b, bn_c_m, bn_c_v,
           chatt_w1, chatt_w2, sa_w, _trace=False):
    if "nc" not in _cache:
        _cache["nc"] = _build_program()
    nc = _cache["nc"]

    consts = _host_consts(
        np.asarray(ca1_w, np.float32), np.asarray(ca1_b, np.float32),
        np.asarray(bn_a_g, np.float32), np.asarray(bn_a_b, np.float32),
        np.asarray(bn_a_m, np.float32), np.asarray(bn_a_v, np.float32),
        np.asarray(ca2_w, np.float32), np.asarray(ca2_b, np.float32),
        np.asarray(bn_b_g, np.float32), np.asarray(bn_b_b, np.float32),
        np.asarray(bn_b_m, np.float32), np.asarray(bn_b_v, np.float32),
        np.asarray(conv1_w, np.float32), np.asarray(conv1_b, np.float32),
        np.asarray(bn_c_g, np.float32), np.asarray(bn_c_b, np.float32),
        np.asarray(bn_c_m, np.float32), np.asarray(bn_c_v, np.float32),
        np.asarray(chatt_w1, np.float32), np.asarray(chatt_w2, np.float32),
        np.asarray(sa_w, np.float32))

    fv = np.asarray(f_vi, np.float32).reshape(N, C, P)
    fi = np.asarray(f_ir, np.float32).reshape(N, C, P)
    in_maps = []
    for i in range(NCORES):
        m = dict(consts)
        m["f_vi"] = np.ascontiguousarray(fv[i * SPC:(i + 1) * SPC])
        m["f_ir"] = np.ascontiguousarray(fi[i * SPC:(i + 1) * SPC])
        in_maps.append(m)

    res = bass_utils.run_bass_kernel_spmd(nc, in_maps, core_ids=list(range(NCORES)),
                                          trace=_trace)
    if _trace:
        _cache["last_trace"] = res
    out = np.concatenate([res.results[i]["out"] for i in range(NCORES)], axis=0)
    return out.reshape(N, C, H, W)
